# revision 1
# baseline (speedup 1.0000x reference)
"""CrossViewTransformer Trainium2 kernel.

Shards batch B=4 x row-halves over 8 NeuronCores (pure data parallel,
one program, per-core data). Per core:
  q = Wq @ cross_ext          (C/8=32, 2176)   fp16 hi/lo split MMs (exact-ish)
  k = Wk @ front_x            (32, 4096)        same
  energy[j,i] = <q_j, k_i>    via single K=128 fp16 [qh;ql;qh;ql]x[kh;kh;kl;kl] MM
  argmax/max over keys on DVE: reduce_max + sum((e==M)*iota) per psum chunk
  v = Wv @ x_hat              fp16 single MM, gathered by argmax via gpsimd ap_gather
  conv3x3([front_x; T]) * S + front_x   fp16 MMs, 9 taps x 4 ci-blocks x 2 o-blocks
"""
import sys

sys.path.insert(0, "/opt/trn_rl_repo")
import numpy as np  # noqa: E402
import concourse.bacc as bacc  # noqa: E402
import concourse.mybir as mybir  # noqa: E402
import concourse.tile as tile  # noqa: E402
from concourse import bass_utils  # noqa: E402

dt = mybir.dt
ALU = mybir.AluOpType
AX = mybir.AxisListType

B, C, H, W = 4, 256, 64, 64
C8 = C // 8            # 32
HWF = H * W            # 4096 keys
RH = H // 2            # 32 out rows per core
EXTR = RH + 2          # 34 ext rows (1 halo/zero row each side)
EXTQ = EXTR * W        # 2176 ext queries
NBLK = EXTQ // 128     # 17 query blocks
OUTP = RH * W          # 2048 out positions
WP = W + 2             # 66 padded width
CATW = EXTR * WP       # 2244 padded cat row-major size
NCHUNK = 4             # energy chunks of 1024 keys
VCOLS = HWF + 4        # v buffer cols (idx HWF -> zero column), 4-aligned

_CACHED = {}
DEBUG_DUMPS = False


def _build(has_bqk: bool, has_bv: bool):
    key = (has_bqk, has_bv)
    if key in _CACHED:
        return _CACHED[key]
    nc = bacc.Bacc("TRN2", debug=False)

    fx_d = nc.dram_tensor("fx", (2, 128, HWF), dt.float32, kind="ExternalInput")
    cx_d = nc.dram_tensor("cx", (2, 128, EXTQ), dt.float32, kind="ExternalInput")
    fpad_d = nc.dram_tensor("fpad", (2, 128, EXTR, W), dt.float32, kind="ExternalInput")
    xh_d = nc.dram_tensor("xh", (2, 128, HWF), dt.float32, kind="ExternalInput")
    wq_d = nc.dram_tensor("wq", (C8, C), dt.float32, kind="ExternalInput")
    wk_d = nc.dram_tensor("wk", (C8, C), dt.float32, kind="ExternalInput")
    wv_d = nc.dram_tensor("wv", (C, C), dt.float32, kind="ExternalInput")
    wf_d = nc.dram_tensor("wf", (C, 2 * C * 9), dt.float32, kind="ExternalInput")
    bq_d = nc.dram_tensor("bq", (C8, 1), dt.float32, kind="ExternalInput")
    bk_d = nc.dram_tensor("bk", (C8, 1), dt.float32, kind="ExternalInput")
    bv_d = nc.dram_tensor("bv", (128, 2), dt.float32, kind="ExternalInput")
    bf_d = nc.dram_tensor("bf", (128, 2), dt.float32, kind="ExternalInput")
    mask_d = nc.dram_tensor("mask", (128, NBLK), dt.float32, kind="ExternalInput")
    amask_d = nc.dram_tensor("amask", (128, NBLK), dt.float32, kind="ExternalInput")
    id_d = nc.dram_tensor("ident", (128, 128), dt.float32, kind="ExternalInput")

    out_d = nc.dram_tensor("out", (2, 128, OUTP), dt.float32, kind="ExternalOutput")
    dbg_arg_d = nc.dram_tensor("dbg_arg", (128, NBLK), dt.float32, kind="ExternalOutput")
    dbg_s_d = nc.dram_tensor("dbg_s", (128, NBLK), dt.float32, kind="ExternalOutput")

    if DEBUG_DUMPS:
        dump_s128_d = nc.dram_tensor("dump_s128", (128, EXTQ), dt.float32, kind="ExternalOutput")
        dump_tg_d = nc.dram_tensor("dump_tg", (128, EXTQ), dt.float32, kind="ExternalOutput")
        dump_cat2_d = nc.dram_tensor("dump_cat2", (128, CATW), dt.float32, kind="ExternalOutput")
        dump_cat0_d = nc.dram_tensor("dump_cat0", (128, CATW), dt.float32, kind="ExternalOutput")
        dump_conv_d = nc.dram_tensor("dump_conv", (128, 512), dt.float32, kind="ExternalOutput")
        dump_idxw_d = nc.dram_tensor("dump_idxw", (128, EXTQ // 16), dt.float32, kind="ExternalOutput")
        dump_vbuf_d = nc.dram_tensor("dump_vbuf", (128, VCOLS), dt.float32, kind="ExternalOutput")

    with tile.TileContext(nc) as tc:
        _body(nc, tc, locals(), has_bqk, has_bv)
    nc.compile()
    _CACHED[key] = nc
    return nc


def _body(nc, tc, T, has_bqk, has_bv):
    F32, F16, I16 = dt.float32, dt.float16, dt.int16

    with tc.tile_pool(name="dramscr", bufs=1, space="DRAM") as DR, \
         tc.tile_pool(name="persist", bufs=1) as P, \
         tc.tile_pool(name="stream", bufs=2) as S, \
         tc.tile_pool(name="tgpool", bufs=1) as TG, \
         tc.tile_pool(name="pse", bufs=2, space="PSUM") as PSE, \
         tc.tile_pool(name="psb", bufs=4, space="PSUM") as PSB:

        ident = P.tile([128, 128], F32, tag="ident")
        nc.sync.dma_start(ident[:, :], T["id_d"].ap())

        # ---------- weight transposes ----------
        # wqkT layout [128, 8*32]: (which q/k, cb, hi/lo) -> col ((which*2+cb)*2+hl)*32
        wqkT = P.tile([128, 8 * C8], F16, tag="wqkT")
        wvT = P.tile([128, 4 * 128], F16, tag="wvT")
        with tc.tile_pool(name="wqr", bufs=1) as WQR:
            wraw = WQR.tile([C8, 2 * C], F32, tag="wqkraw")
            nc.sync.dma_start(wraw[:, 0:C], T["wq_d"].ap())
            nc.sync.dma_start(wraw[:, C:2 * C], T["wk_d"].ap())
            for which in range(2):
                for cb in range(2):
                    pt = PSB.tile([128, 512], F32, tag="ps512")
                    nc.tensor.transpose(pt[:, 0:C8], wraw[:, which * C + cb * 128:which * C + (cb + 1) * 128],
                                        ident[0:C8, 0:C8])
                    base = ((which * 2 + cb) * 2) * C8
                    nc.scalar.copy(wqkT[:, base:base + C8], pt[:, 0:C8])
                    nc.vector.tensor_tensor(wqkT[:, base + C8:base + 2 * C8], pt[:, 0:C8],
                                            wqkT[:, base:base + C8], op=ALU.subtract)

            # wvT layout [128, 4*128]: (cb, ob) -> col (cb*2+ob)*128 ; fp16 plain
            vraw = WQR.tile([128, 2 * C], F32, tag="wvraw")
            nc.sync.dma_start(vraw[:, 0:C], T["wv_d"].ap()[0:128, :])
            nc.sync.dma_start(vraw[:, C:2 * C], T["wv_d"].ap()[128:256, :])
            for ob in range(2):
                for cb in range(2):
                    pt = PSB.tile([128, 512], F32, tag="ps512")
                    nc.tensor.transpose(pt[:, 0:128], vraw[:, ob * C + cb * 128:ob * C + (cb + 1) * 128],
                                        ident[:, :])
                    nc.scalar.copy(wvT[:, (cb * 2 + ob) * 128:(cb * 2 + ob + 1) * 128], pt[:, 0:128])

        # wfT layout [128, 72*128]: (cb4, tap, ob) -> col ((cb4*9+tap)*2+ob)*128 ; fp16
        wfT = P.tile([128, 72 * 128], F16, tag="wfT")
        with tc.tile_pool(name="wfraw_pool", bufs=1) as WR:
            fraw = WR.tile([128, 2 * 2 * C * 9], F32, tag="wfraw")
            nc.sync.dma_start(fraw[:, 0:2 * C * 9], T["wf_d"].ap()[0:128, :])
            nc.sync.dma_start(fraw[:, 2 * C * 9:], T["wf_d"].ap()[128:256, :])
            for ob in range(2):
                for cb4 in range(4):
                    for tap in range(9):
                        pt = PSB.tile([128, 512], F32, tag="ps512")
                        # columns f = ci*9 + tap for ci in [cb4*128, cb4*128+128)
                        fview = fraw[:, ob * (2 * C * 9):(ob + 1) * (2 * C * 9)]
                        fview = fview.rearrange("p (ci tap) -> p ci tap", tap=9)
                        in_ap = fview[:, cb4 * 128:(cb4 + 1) * 128, tap]
                        nc.tensor.transpose(pt[:, 0:128], in_ap, ident[:, :])
                        col = ((cb4 * 9 + tap) * 2 + ob) * 128
                        nc.scalar.copy(wfT[:, col:col + 128], pt[:, 0:128])

        # ---------- load + split activations (scoped pool AB) ----------
        AB_cm = tc.tile_pool(name="actsplit", bufs=1)
        AB = AB_cm.__enter__()
        cxh = AB.tile([128, 2, EXTQ], F16, tag="cxh")
        cxl = AB.tile([128, 2, EXTQ], F16, tag="cxl")
        fxh = AB.tile([128, 2, HWF], F16, tag="fxh")
        fxl = AB.tile([128, 2, HWF], F16, tag="fxl")
        with tc.tile_pool(name="actraw", bufs=1) as AR:
            for cb in range(2):
                t = AR.tile([128, HWF], F32, tag="actr")
                nc.sync.dma_start(t[:, 0:EXTQ], T["cx_d"].ap()[cb])
                nc.scalar.copy(cxh[:, cb], t[:, 0:EXTQ])
                nc.vector.tensor_tensor(cxl[:, cb], t[:, 0:EXTQ], cxh[:, cb], op=ALU.subtract)
            for cb in range(2):
                t = AR.tile([128, HWF], F32, tag="actr")
                nc.sync.dma_start(t[:, :], T["fx_d"].ap()[cb])
                nc.scalar.copy(fxh[:, cb], t[:, :])
                nc.vector.tensor_tensor(fxl[:, cb], t[:, :], fxh[:, cb], op=ALU.subtract)

        SM = P.tile([128, 256], F32, tag="smalls")
        SI = P.tile([128, 160], I16, tag="ints")
        bqs = SM[0:C8, 225:227]
        nc.sync.dma_start(bqs[:, 0:1], T["bq_d"].ap())
        nc.sync.dma_start(bqs[:, 1:2], T["bk_d"].ap())
        bvs = SM[:, 227:229]
        nc.sync.dma_start(bvs[:, :], T["bv_d"].ap())
        bfs = SM[:, 229:231]
        nc.sync.dma_start(bfs[:, :], T["bf_d"].ap())

        # ---------- q, k (fp16 hi/lo x hi/lo accumulation) ----------
        qstack = P.tile([128, EXTQ], F16, tag="qstack")
        kstack = P.tile([128, HWF], F16, tag="kstack")

        def qk_mm(which, xh_t, xl_t, npos, stack, hrows, lrows):
            nchunks = (npos + 511) // 512
            for ch in range(nchunks):
                n0, n1 = ch * 512, min((ch + 1) * 512, npos)
                pq = PSB.tile([C8, 512], F32, tag="ps512")
                first = True
                for cb in range(2):
                    for wsplit in range(2):
                        wcol = ((which * 2 + cb) * 2 + wsplit) * C8
                        for xs, xt in ((0, xh_t), (1, xl_t)):
                            nc.tensor.matmul(
                                pq[:, 0:n1 - n0],
                                wqkT[:, wcol:wcol + C8],
                                xt[:, cb, n0:n1],
                                start=first, stop=(cb == 1 and wsplit == 1 and xs == 1))
                            first = False
                if has_bqk:
                    nc.vector.tensor_scalar(
                        out=stack[hrows[0]:hrows[0] + C8, n0:n1], in0=pq[:, 0:n1 - n0],
                        scalar1=bqs[:, which:which + 1], scalar2=None, op0=ALU.add)
                    nc.vector.scalar_tensor_tensor(
                        stack[lrows[0]:lrows[0] + C8, n0:n1], pq[:, 0:n1 - n0],
                        bqs[:, which:which + 1], stack[hrows[0]:hrows[0] + C8, n0:n1],
                        op0=ALU.add, op1=ALU.subtract)
                else:
                    nc.scalar.copy(stack[hrows[0]:hrows[0] + C8, n0:n1], pq[:, 0:n1 - n0])
                    nc.vector.scalar_tensor_tensor(
                        stack[lrows[0]:lrows[0] + C8, n0:n1], pq[:, 0:n1 - n0],
                        0.0, stack[hrows[0]:hrows[0] + C8, n0:n1],
                        op0=ALU.add, op1=ALU.subtract)
                for extra in hrows[1:]:
                    nc.vector.tensor_copy(stack[extra:extra + C8, n0:n1],
                                          stack[hrows[0]:hrows[0] + C8, n0:n1])
                for extra in lrows[1:]:
                    nc.vector.tensor_copy(stack[extra:extra + C8, n0:n1],
                                          stack[lrows[0]:lrows[0] + C8, n0:n1])

        # qstack rows: [qh, ql, qh, ql] ; kstack rows: [kh, kh, kl, kl]
        qk_mm(0, cxh, cxl, EXTQ, qstack, hrows=(0, 64), lrows=(32, 96))
        qk_mm(1, fxh, fxl, HWF, kstack, hrows=(0, 32), lrows=(64, 96))

        # ---------- v + gather ----------
        vbuf0 = P.tile([128, VCOLS], F32, tag="vbuf0")
        vbuf1 = P.tile([128, VCOLS], F32, tag="vbuf1")
        vbufs = (vbuf0, vbuf1)
        nc.vector.memset(vbuf0[:, HWF:VCOLS], 0.0)
        nc.vector.memset(vbuf1[:, HWF:VCOLS], 0.0)
        for ch in range(HWF // 512):
            xr = S.tile([128, 2, 512], F32, tag="xraw")
            nc.sync.dma_start(xr[:, 0], T["xh_d"].ap()[0][:, ch * 512:(ch + 1) * 512])
            nc.sync.dma_start(xr[:, 1], T["xh_d"].ap()[1][:, ch * 512:(ch + 1) * 512])
            xc = S.tile([128, 2, 512], F16, tag="xc16")
            nc.scalar.copy(xc[:, 0], xr[:, 0])
            nc.scalar.copy(xc[:, 1], xr[:, 1])
            for ob in range(2):
                pv = PSB.tile([128, 512], F32, tag="ps512")
                for cb in range(2):
                    nc.tensor.matmul(pv[:, :], wvT[:, (cb * 2 + ob) * 128:(cb * 2 + ob + 1) * 128],
                                     xc[:, cb, :],
                                     start=(cb == 0), stop=(cb == 1))
                if has_bv:
                    nc.vector.tensor_scalar(
                        out=vbufs[ob][:, ch * 512:(ch + 1) * 512], in0=pv[:, :],
                        scalar1=bvs[:, ob:ob + 1], scalar2=None, op0=ALU.add)
                else:
                    nc.scalar.copy(vbufs[ob][:, ch * 512:(ch + 1) * 512], pv[:, :])

        AB_cm.__exit__(None, None, None)

        # ---------- energy + argmax ----------
        iot = P.tile([128, HWF], I16, tag="iota")
        nc.gpsimd.iota(iot[:, :], pattern=[[1, HWF]], base=0, channel_multiplier=0)
        Mg = SM[:, 0:17]
        Ag = SM[:, 17:34]
        mch = SM[:, 34:102].rearrange("p (c b) -> p c b", c=NCHUNK)
        ach = SM[:, 102:170].rearrange("p (c b) -> p c b", c=NCHUNK)
        sel = SM[:, 221:225]
        scratch = S.tile([128, 1024], F32, tag="escr")
        for b in range(NBLK):
            for c in range(NCHUNK):
                pe = PSE.tile([128, 1024], F32, tag="pe")
                nc.tensor.matmul(pe[:, 0:512], qstack[:, b * 128:(b + 1) * 128],
                                 kstack[:, c * 1024:c * 1024 + 512], start=True, stop=True)
                nc.tensor.matmul(pe[:, 512:1024], qstack[:, b * 128:(b + 1) * 128],
                                 kstack[:, c * 1024 + 512:(c + 1) * 1024], start=True, stop=True)
                nc.vector.tensor_reduce(mch[:, c, b:b + 1], pe[:, :], axis=AX.X, op=ALU.max)
                nc.vector.scalar_tensor_tensor(
                    scratch[:, :], pe[:, :], mch[:, c, b:b + 1],
                    iot[:, c * 1024:(c + 1) * 1024],
                    op0=ALU.is_equal, op1=ALU.mult, accum_out=ach[:, c, b:b + 1])
            nc.vector.tensor_reduce(Mg[:, b:b + 1], mch[:, :, b], axis=AX.X, op=ALU.max)
            nc.vector.scalar_tensor_tensor(
                sel[:, :], mch[:, :, b], Mg[:, b:b + 1], ach[:, :, b],
                op0=ALU.is_equal, op1=ALU.mult, accum_out=Ag[:, b:b + 1])
        nc.sync.dma_start(T["dbg_s_d"].ap(), Mg[:, :])
        nc.sync.dma_start(T["dbg_arg_d"].ap(), Ag[:, :])

        # masked arg: arg2 = arg*mask + amask  (amask = (1-mask)*HWF)
        maskt = SM[:, 170:187]
        amaskt = SM[:, 187:204]
        nc.sync.dma_start(maskt[:, :], T["mask_d"].ap())
        nc.sync.dma_start(amaskt[:, :], T["amask_d"].ap())
        arg2 = SM[:, 204:221]
        nc.vector.tensor_tensor(arg2[:, :], Ag[:, :], maskt[:, :], op=ALU.mult)
        nc.vector.tensor_tensor(arg2[:, :], arg2[:, :], amaskt[:, :], op=ALU.add)

        # transpose [Mg | arg2] -> [34, 128]; relayout via DRAM with fat runs
        ptx = PSB.tile([128, 512], F32, tag="ps512")
        nc.tensor.transpose(ptx[0:NBLK, 0:128], Mg[:, :], ident[:, :])
        srow_stage = S.tile([NBLK, 128], F32, tag="stage")
        nc.scalar.copy(srow_stage[:, :], ptx[0:NBLK, 0:128])
        ptx2 = PSB.tile([128, 512], F32, tag="ps512")
        nc.tensor.transpose(ptx2[0:NBLK, 0:128], arg2[:, :], ident[:, :])
        argT16 = S.tile([NBLK, 128], I16, tag="argT16")
        nc.vector.tensor_copy(argT16[:, :], ptx2[0:NBLK, 0:128])
        # reorder free (t,p0)->(p0,t): argTr[b, p0*8+t] = argT16[b, 16t+p0]
        argTr = S.tile([NBLK, 128], I16, tag="argTr")
        nc.vector.tensor_copy(
            argTr[:, :],
            argT16[:, :].rearrange("b (t p0) -> b p0 t", t=8, p0=16))
        # DRAM wrap: wrap[p0*136 + b*8 + t] = argTr[b, p0*8+t]
        wrap_t = DR.tile([EXTQ], I16, tag="wrapl")
        nc.sync.dma_start(
            wrap_t[:].rearrange("(p0 b t) -> b p0 t", p0=16, t=8),
            argTr[:, :].rearrange("b (p0 t) -> b p0 t", p0=16))
        idxw = SI[:, 0:EXTQ // 16]
        nc.sync.dma_start(idxw[0:16, :], wrap_t[:].rearrange("(p0 s) -> p0 s", p0=16))
        for g in range(1, 8):
            nc.sync.dma_start(idxw[16 * g:16 * (g + 1), :], idxw[0:16, :])

        # S row: contiguous dram roundtrip + broadcast
        srow_t = DR.tile([EXTQ], F32, tag="srowd")
        nc.sync.dma_start(srow_t[:].rearrange("(b p) -> b p", p=128), srow_stage[:, :])
        srow = P.tile([1, EXTQ], F32, tag="srow")
        nc.sync.dma_start(srow[:, :], srow_t[:].rearrange("q -> () q"))
        s128 = P.tile([128, EXTQ], F32, tag="s128")
        nc.gpsimd.partition_broadcast(s128[:, :], srow[:, :])
        if DEBUG_DUMPS:
            nc.sync.dma_start(T["dump_s128_d"].ap(), s128[:, :])

        # cat buffer: 4 ci-blocks [128, CATW] fp16, zero-padded
        cats = []
        for cb4 in range(4):
            ct = P.tile([128, CATW], F16, tag=f"cat{cb4}")
            nc.vector.memset(ct[:, :], 0.0)
            cats.append(ct)
        # front part from fpad dram
        for cb in range(2):
            with tc.tile_pool(name=f"fpr{cb}", bufs=1) as FR:
                t = FR.tile([128, EXTR, W], F32, tag="fpr")
                nc.sync.dma_start(t[:, :, :], T["fpad_d"].ap()[cb])
                nc.vector.tensor_copy(
                    cats[cb][:, :].rearrange("p (r wp) -> p r wp", wp=WP)[:, :, 1:W + 1],
                    t[:, :, :])
        # T part via gather
        for ob in range(2):
            tg = TG.tile([128, EXTQ], F32, tag="tg")
            nc.gpsimd.ap_gather(tg[:, :], vbufs[ob][:, 0:VCOLS], idxw[:, :],
                                channels=128, num_elems=VCOLS, d=1, num_idxs=EXTQ)
            nc.vector.tensor_copy(
                cats[2 + ob][:, :].rearrange("p (r wp) -> p r wp", wp=WP)[:, :, 1:W + 1],
                tg[:, :].rearrange("p (r w) -> p r w", w=W))
            if DEBUG_DUMPS and ob == 0:
                nc.sync.dma_start(T["dump_tg_d"].ap(), tg[:, :])

        if DEBUG_DUMPS:
            nc.sync.dma_start(T["dump_vbuf_d"].ap(), vbuf0[:, :])
            iwf = S.tile([128, EXTQ // 16], F32, tag="stage")
            nc.vector.tensor_copy(iwf[:, :], idxw[:, :])
            nc.sync.dma_start(T["dump_idxw_d"].ap(), iwf[:, :])
            dc = TG.tile([128, CATW], F32, tag="tg")
            nc.vector.tensor_copy(dc[:, :], cats[2][:, :])
            nc.sync.dma_start(T["dump_cat2_d"].ap(), dc[:, :])
            nc.vector.tensor_copy(dc[:, :], cats[0][:, :])
            nc.sync.dma_start(T["dump_cat0_d"].ap(), dc[:, :])
        # ---------- conv + assembly ----------
        for ob in range(2):
            pcs = []
            for g in range(4):
                pc = PSB.tile([128, 512], F32, tag="ps512")
                pcs.append((g, pc))
            for cb4 in range(4):
                for tap in range(9):
                    dy, dx = tap // 3, tap % 3
                    col = ((cb4 * 9 + tap) * 2 + ob) * 128
                    first = (cb4 == 0 and tap == 0)
                    last = (cb4 == 3 and tap == 8)
                    for g, pc in pcs:
                        catv = cats[cb4][:, :].rearrange("p (r wp) -> p r wp", wp=WP)
                        rhs = catv[:, g * 8 + dy:g * 8 + dy + 8, dx:dx + W]
                        nc.tensor.matmul(pc[:, :], wfT[:, col:col + 128], rhs,
                                         start=first, stop=last)
            if True:
                for g, pc in pcs:
                    if DEBUG_DUMPS and ob == 0 and g == 0:
                        dstage = S.tile([128, 512], F32, tag="stage")
                        nc.scalar.copy(dstage[:, :], pc[:, :])
                        nc.sync.dma_start(T["dump_conv_d"].ap(), dstage[:, :])
                    stage = S.tile([128, 512], F32, tag="stage")
                    nc.vector.scalar_tensor_tensor(
                        stage[:, :], pc[:, :], bfs[:, ob:ob + 1],
                        s128[:, W + g * 512:W + (g + 1) * 512],
                        op0=ALU.add, op1=ALU.mult)
                    fcatv = cats[ob][:, :].rearrange("p (r wp) -> p r wp", wp=WP)
                    front_mid = fcatv[:, g * 8 + 1:g * 8 + 9, 1:1 + W]
                    nc.vector.tensor_tensor(stage[:, :], stage[:, :], front_mid, op=ALU.add)
                    nc.sync.dma_start(T["out_d"].ap()[ob][:, g * 512:(g + 1) * 512],
                                      stage[:, :])


def _prep_core_inputs(inputs, core):
    b, half = core // 2, core % 2
    r0 = half * RH
    fx = np.ascontiguousarray(inputs["front_x"][b].reshape(2, 128, HWF))
    xh = np.ascontiguousarray(inputs["front_x_hat"][b].reshape(2, 128, HWF))

    def ext_rows(x):  # (C,H,W) -> (2,128,EXTR,W) with zero boundary row
        out = np.zeros((C, EXTR, W), x.dtype)
        lo, hi = r0 - 1, r0 + RH + 1
        slo, dlo = max(lo, 0), max(lo, 0) - lo
        shi = min(hi, H)
        out[:, dlo:dlo + shi - slo] = x[:, slo:shi]
        return np.ascontiguousarray(out.reshape(2, 128, EXTR, W))

    cxe = ext_rows(inputs["cross_x"][b])
    fpe = ext_rows(inputs["front_x"][b])
    # T-halo mask over ext queries: invalid rows are the zero-padded boundary
    valid = np.ones((EXTR, W), np.float32)
    if r0 == 0:
        valid[0] = 0.0
    if r0 + RH == H:
        valid[-1] = 0.0
    vq = valid.reshape(EXTQ)
    mask = np.empty((128, NBLK), np.float32)
    for blk in range(NBLK):
        mask[:, blk] = vq[blk * 128:(blk + 1) * 128]
    amask = (1.0 - mask) * HWF
    return {
        "fx": fx, "cx": cxe.reshape(2, 128, EXTQ), "fpad": fpe, "xh": xh,
        "wq": inputs["Wq"], "wk": inputs["Wk"], "wv": inputs["Wv"],
        "wf": np.ascontiguousarray(inputs["Wf"].reshape(C, 2 * C * 9)),
        "bq": inputs["bq"].reshape(C8, 1), "bk": inputs["bk"].reshape(C8, 1),
        "bv": np.ascontiguousarray(inputs["bv"].reshape(2, 128).T),
        "bf": np.ascontiguousarray(inputs["bf"].reshape(2, 128).T),
        "mask": mask, "amask": amask,
        "ident": np.eye(128, dtype=np.float32),
    }


LAST_RES = None


def kernel(_trace=False, **inputs):
    global LAST_RES
    inputs = {k: np.asarray(v, dtype=np.float32) for k, v in inputs.items()}
    has_bqk = bool(np.any(inputs["bq"]) or np.any(inputs["bk"]))
    has_bv = bool(np.any(inputs["bv"]))
    nc = _build(has_bqk, has_bv)
    in_maps = [_prep_core_inputs(inputs, core) for core in range(8)]
    kw = {"trace": True} if _trace else {}
    res = bass_utils.run_bass_kernel_spmd(nc, in_maps, core_ids=list(range(8)), **kw)
    LAST_RES = res
    out = np.empty((B, C, H, W), np.float32)
    for core in range(8):
        b, half = core // 2, core % 2
        o = res.results[core]["out"].reshape(C, RH, W)
        out[b, :, half * RH:(half + 1) * RH, :] = o
    return out


if __name__ == "__main__":
    rng = np.random.default_rng(0)
    ins = {
        "front_x": rng.standard_normal((B, C, H, W)).astype(np.float32),
        "cross_x": rng.standard_normal((B, C, H, W)).astype(np.float32),
        "front_x_hat": rng.standard_normal((B, C, H, W)).astype(np.float32),
        "Wq": (rng.standard_normal((C8, C)) / 16).astype(np.float32),
        "bq": np.zeros((C8,), np.float32),
        "Wk": (rng.standard_normal((C8, C)) / 16).astype(np.float32),
        "bk": np.zeros((C8,), np.float32),
        "Wv": (rng.standard_normal((C, C)) / 16).astype(np.float32),
        "bv": np.zeros((C,), np.float32),
        "Wf": (rng.standard_normal((C, 2 * C, 3, 3)) / 68).astype(np.float32),
        "bf": np.zeros((C,), np.float32),
    }
    out = kernel(**ins)
    print("kernel ran, out shape", out.shape, "std", out.std())

